# revision 26
# baseline (speedup 1.0000x reference)
"""AnalyticFlow drift kernel for 8 TRN2 NeuronCores.

Math (per reference): eta = softmax_n(a_b * (x.gt_n - (t_b/2)*||gt_n||^2)),
drift = (eta @ gt - x) / (1 - t/999).  The per-row affine terms of log_p drop
out of the row softmax; a_b = tt/sig^2 > 0.

Sharding: gt_images split along N across 8 cores (2500 rows each).  Each core
runs a flash-style local-max softmax over its shard, then cores combine with
an AllReduce(max) of the row maxima and a ReduceScatter(add) of
[y_partial | S_partial] rows; core c finishes rows 16c:16c+16 of the batch.
"""

import sys

for p in ("/opt/trn_rl_repo", "/opt/pypackages"):
    if p not in sys.path:
        sys.path.append(p)

import numpy as np

import concourse.bass as bass
import concourse.tile as tile
from concourse import mybir
from concourse.bass import ds
from concourse.bass_utils import run_bass_kernel_spmd
from concourse.masks import make_identity

F32 = mybir.dt.float32
F32R = mybir.dt.float32r
AX = mybir.AxisListType.X
OP = mybir.AluOpType
AF = mybir.ActivationFunctionType

B = 128
D = 3072
N = 20000
NCORES = 8
NSH = N // NCORES  # 2500
NSH_PAD = 2560  # shard padded to 20x128 rows host-side (pad rows zero)
BR = B // NCORES  # 16 output rows per core
T_SCHEDULE = 999.0

# n-tiles per core: 4x512 + 452
TILE_N = 512
TILES = [512, 512, 512, 512, 452]
ND = D // 128  # 24 d-chunks
NDD = D // 512  # 6 output column groups for GEMM2


def build_graph(split_waits=True):
    nc = bass.Bass()

    x_ext = nc.declare_dram_parameter("x", [B, D], F32, isOutput=False)
    t_ext = nc.declare_dram_parameter("t", [B, 1], F32, isOutput=False)
    gt_ext = nc.declare_dram_parameter("gt", [NSH_PAD, D], F32R, isOutput=False)
    out_ext = nc.declare_dram_parameter("out", [BR, D], F32, isOutput=True)

    # collective bounce buffers
    cc_m_in = nc.dram_tensor("cc_m_in", [B, 1], F32)
    cc_m_out = nc.dram_tensor("cc_m_out", [B, 1], F32, addr_space="Shared")
    rs_in = nc.dram_tensor("rs_in", [B, D + 1], F32)
    rs_out = nc.dram_tensor("rs_out", [BR, D + 1], F32)
    groups = [list(range(NCORES))]

    with tile.TileContext(nc) as tc:
        with (
            tc.tile_pool(name="const", bufs=1) as cpool,
            tc.tile_pool(name="gin", bufs=5) as gpool,
            tc.tile_pool(name="gtt", bufs=1) as gttpool,
            tc.tile_pool(name="pp", bufs=5) as ppool,
            tc.tile_pool(name="ptp", bufs=2) as ptpool,
            tc.tile_pool(name="small", bufs=10) as spool,
            tc.tile_pool(name="ps_t", bufs=2, space="PSUM") as ps_t,
            tc.tile_pool(name="ps_x", bufs=2, space="PSUM") as ps_x,
            tc.tile_pool(name="ps_z", bufs=2, space="PSUM") as ps_z,
            tc.tile_pool(name="ps_y", bufs=2, space="PSUM") as ps_y,
        ):
            # ---- constants / setup ----
            ident = cpool.tile([128, 128], F32, tag="ident")
            make_identity(nc, ident[:])
            identr = cpool.tile([128, 128], F32R, tag="identr")
            nc.vector.tensor_copy(out=identr[:], in_=ident[:])

            # stage x through DVE so the first transpose has a single-proc wait
            # (walrus allows only 1 sync wait on a self-loading f32r matmult).
            # y_resc is dead until the finale, so borrow it as the landing pad.
            sq_scr = cpool.tile([128, D], F32, tag="sq_scr")
            y_resc = cpool.tile([B, D + 1], F32, tag="y_resc")
            x_sb = cpool.tile([B, D], F32R, tag="x_sb")
            nc.sync.dma_start(y_resc[:, :D], x_ext[:])
            nc.vector.tensor_copy(out=x_sb[:], in_=y_resc[:, :D])
            t_sb = cpool.tile([B, 1], F32, tag="t_sb")
            nc.sync.dma_start(t_sb[:], t_ext[:])

            tt = cpool.tile([B, 1], F32, tag="tt")
            nc.vector.tensor_scalar_mul(tt[:], t_sb[:], 1.0 / T_SCHEDULE)
            sig = cpool.tile([B, 1], F32, tag="sig")
            nc.vector.tensor_scalar(sig[:], tt[:], -1.0, 1.0, OP.mult, OP.add)
            sig2 = cpool.tile([B, 1], F32, tag="sig2")
            nc.vector.tensor_mul(sig2[:], sig[:], sig[:])
            rsig2 = cpool.tile([B, 1], F32, tag="rsig2")
            nc.vector.reciprocal(rsig2[:], sig2[:])
            a_sc = cpool.tile([B, 1], F32, tag="a_sc")
            nc.vector.tensor_mul(a_sc[:], tt[:], rsig2[:])
            neg_a = cpool.tile([B, 1], F32, tag="neg_a")
            nc.vector.tensor_scalar_mul(neg_a[:], a_sc[:], -1.0)
            nht = cpool.tile([B, 1], F32R, tag="nht")  # -t/2 (t = tt here)
            nc.vector.tensor_scalar_mul(nht[:], tt[:], -0.5)

            # xT: chunk d at [:, 128d:128(d+1)] holds x[:, 128d:...]^T
            xT = cpool.tile([128, D], F32R, tag="xT")
            for d in range(ND):
                pst = ps_x.tile([128, 128], F32R, tag="tpx")
                nc.tensor.transpose(pst[:], x_sb[:, 128 * d : 128 * (d + 1)], identr[:])
                nc.vector.tensor_copy(out=xT[:, 128 * d : 128 * (d + 1)], in_=pst[:])

            # aug lhsT: rows 0/1 = (-t/2) over free dim b, other rows zero
            # (f32r can't be memset directly: zero f32 scratch, copy-convert)
            aug_lhsT = cpool.tile([128, 128], F32R, tag="aug")
            nc.vector.memset(sq_scr[:, :TILE_N], 0.0)
            nc.vector.tensor_copy(out=aug_lhsT[:], in_=sq_scr[:, :128])
            nht2 = cpool.tile([B, 2], F32R, tag="nht2")
            nc.vector.tensor_copy(out=nht2[:, 0:1], in_=nht[:])
            nc.vector.tensor_copy(out=nht2[:, 1:2], in_=nht[:])
            pst = ps_x.tile([128, 128], F32R, tag="tpx")
            nc.tensor.transpose(pst[:2, :], nht2[:], identr[:])
            nc.vector.tensor_copy(out=aug_lhsT[:2, :], in_=pst[:2, :])

            # g2 staging: rows 0/1 = g2 hi/lo of current tile, rest stay zero
            g2stage = cpool.tile([128, TILE_N], F32R, tag="g2stage")
            nc.vector.tensor_copy(out=g2stage[:], in_=sq_scr[:, :TILE_N])

            y_sb = cpool.tile([B, D], F32, tag="y_sb")
            nc.vector.memset(y_sb[:], 0.0)

            zmax_run = cpool.tile([B, 1], F32, tag="zmax_run")
            nc.vector.memset(zmax_run[:], -1.0e30)
            s_run = cpool.tile([B, 1], F32, tag="s_run")
            nc.vector.memset(s_run[:], 0.0)

            # ---- main loop over n-tiles ----
            n_off = 0
            for ti, nt in enumerate(TILES):
                nsub = (nt + 127) // 128  # 4
                sub_rows = [min(128, nt - 128 * j) for j in range(nsub)]

                gsub = []
                for j in range(nsub):
                    r = sub_rows[j]
                    if r < 128:
                        # dedicated tile (no pool WAW); full-width DMA reads the
                        # host-side zero padding, so no memset is needed
                        g = cpool.tile([128, D], F32R, tag="gtail")
                    else:
                        g = gpool.tile([128, D], F32R, tag="g")
                    nc.scalar.dma_start(
                        out=g[:, :], in_=gt_ext[n_off + 128 * j : n_off + 128 * (j + 1), :]
                    )
                    gsub.append(g)

                # g2 per subtile -> g2stage row 0
                for j in range(nsub):
                    # unique tiles (never pool-recycled): the square then waits
                    # only on its own gsub DMA, within the 1-wait ACT limit
                    g2c = cpool.tile([128, 1], F32, tag=f"g2c_{ti}_{j}")
                    g2d = cpool.tile([128, 1], F32, tag=f"g2d_{ti}_{j}")
                    nc.scalar.activation(
                        g2d.broadcast_to(gsub[j][:].shape),
                        gsub[j][:].bitcast(F32),
                        AF.Square,
                        accum_out=g2c[:],
                    )
                    g2hl = spool.tile([128, 2], F32R, tag="g2hl")
                    nc.vector.tensor_copy(out=g2hl[:, 0:1], in_=g2c[:])
                    g2lo = spool.tile([128, 1], F32, tag="g2lo")
                    nc.vector.tensor_sub(
                        g2lo[:], g2c[:], g2hl[:, 0:1].bitcast(F32)
                    )
                    nc.vector.tensor_copy(out=g2hl[:, 1:2], in_=g2lo[:])
                    pst = ps_x.tile([128, 128], F32R, tag="tpx")
                    nc.tensor.transpose(pst[:2, :], g2hl[:], identr[:])
                    js = slice(128 * j, 128 * (j + 1))
                    nc.vector.tensor_copy(out=g2stage[:2, js], in_=pst[:2, :])

                # transpose gt tile into gtT strips
                gtT = gttpool.tile([128, ND * TILE_N], F32R, tag="gtT")
                for d in range(ND):
                    for j in range(nsub):
                        pst = ps_t.tile([128, 128], F32R, tag="tpr")
                        nc.tensor.transpose(
                            pst[:], gsub[j][:, 128 * d : 128 * (d + 1)], identr[:]
                        )
                        nc.vector.tensor_copy(
                            out=gtT[:, TILE_N * d + 128 * j : TILE_N * d + 128 * (j + 1)],
                            in_=pst[:],
                        )

                # GEMM1: z = x @ gtT_tile - (t/2) g2   [128, nt]
                zp = ps_z.tile([128, TILE_N], F32, tag="zp")
                for d in range(ND):
                    nc.tensor.matmul(
                        zp[:, :nt],
                        xT[:, 128 * d : 128 * (d + 1)],
                        gtT[:, TILE_N * d : TILE_N * d + nt],
                        start=(d == 0),
                        stop=False,
                    )
                nc.tensor.matmul(
                    zp[:, :nt],
                    aug_lhsT[:],
                    g2stage[:, :nt],
                    start=False,
                    stop=True,
                )

                # online max update
                zmax_t = spool.tile([B, 1], F32, tag="zmax_t")
                nc.vector.tensor_reduce(zmax_t[:], zp[:, :nt], AX, OP.max)
                zmax_new = spool.tile([B, 1], F32, tag="zmax_new")
                nc.vector.tensor_max(zmax_new[:], zmax_run[:], zmax_t[:])
                dm = spool.tile([B, 1], F32, tag="dm")
                nc.vector.tensor_sub(dm[:], zmax_run[:], zmax_new[:])
                nc.vector.tensor_copy(out=zmax_run[:], in_=zmax_new[:])
                f_t = spool.tile([B, 1], F32, tag="f_t")
                nc.scalar.activation(f_t[:], dm[:], AF.Exp, scale=a_sc[:])
                negm = spool.tile([B, 1], F32, tag="negm")
                nc.scalar.activation(negm[:], zmax_new[:], AF.Copy, scale=neg_a[:])

                # p = exp(a*z - a*zmax_new), S_t = rowsum(p)
                p_t = ppool.tile([128, TILE_N], F32R, tag="p_t")
                if nt < TILE_N:
                    nc.vector.tensor_copy(
                        out=p_t[:, nt:], in_=sq_scr[:, : TILE_N - nt]
                    )
                s_t = spool.tile([B, 1], F32, tag="s_t")
                nc.scalar.activation(
                    p_t[:, :nt],
                    zp[:, :nt],
                    AF.Exp,
                    bias=negm[:],
                    scale=a_sc[:],
                    accum_out=s_t[:],
                )
                nc.vector.scalar_tensor_tensor(
                    out=s_run[:],
                    in0=s_run[:],
                    scalar=f_t[:],
                    in1=s_t[:],
                    op0=OP.mult,
                    op1=OP.add,
                )

                # pT chunks
                pT = ptpool.tile([128, TILE_N], F32R, tag="pT")
                for j in range(nsub):
                    pst = ps_t.tile([128, 128], F32R, tag="tpr")
                    nc.tensor.transpose(
                        pst[:], p_t[:, 128 * j : 128 * (j + 1)], identr[:]
                    )
                    nc.vector.tensor_copy(
                        out=pT[:, 128 * j : 128 * (j + 1)], in_=pst[:]
                    )

                # GEMM2 + online accumulate: y = y*f + p_tile @ gt_tile
                for dd in range(NDD):
                    yp = ps_y.tile([128, 512], F32, tag="yp")
                    for j in range(nsub):
                        nc.tensor.matmul(
                            yp[:],
                            pT[:, 128 * j : 128 * (j + 1)],
                            gsub[j][:, 512 * dd : 512 * (dd + 1)],
                            start=(j == 0),
                            stop=(j == nsub - 1),
                        )
                    nc.vector.scalar_tensor_tensor(
                        out=y_sb[:, 512 * dd : 512 * (dd + 1)],
                        in0=y_sb[:, 512 * dd : 512 * (dd + 1)],
                        scalar=f_t[:],
                        in1=yp[:],
                        op0=OP.mult,
                        op1=OP.add,
                    )

                n_off += nt

            # ---- cross-core combine ----
            m_c = cpool.tile([B, 1], F32, tag="m_c")
            nc.vector.tensor_mul(m_c[:], a_sc[:], zmax_run[:])
            nc.sync.dma_start(cc_m_in[:], m_c[:])
            nc.gpsimd.collective_compute(
                "AllReduce",
                OP.max,
                ins=[cc_m_in[:]],
                outs=[cc_m_out[:]],
                replica_groups=groups,
            )
            m_g = cpool.tile([B, 1], F32, tag="m_g")
            nc.sync.dma_start(m_g[:], cc_m_out[:])

            dmg = cpool.tile([B, 1], F32, tag="dmg")
            nc.vector.tensor_sub(dmg[:], m_c[:], m_g[:])
            f_c = cpool.tile([B, 1], F32, tag="f_c")
            nc.scalar.activation(f_c[:], dmg[:], AF.Exp)

            # write the rescaled y into sq_scr (ACT-only history keeps the
            # sync-wait count within the ISA limit); S goes via a small tile
            nc.vector.tensor_scalar_mul(sq_scr[:], y_sb[:], f_c[:])
            s_scaled = cpool.tile([B, 1], F32, tag="s_scaled")
            nc.vector.tensor_scalar_mul(s_scaled[:], s_run[:], f_c[:])
            nc.sync.dma_start(rs_in[:, :D], sq_scr[:])
            nc.sync.dma_start(rs_in[:, D : D + 1], s_scaled[:])
            nc.gpsimd.collective_compute(
                "ReduceScatter",
                OP.add,
                ins=[rs_in[:]],
                outs=[rs_out[:]],
                replica_groups=groups,
            )
            fin = y_resc[:BR, :]  # y_resc is dead after rs_in DMA
            nc.sync.dma_start(fin[:], rs_out[:])

            # this core's rows
            rv = nc.sync.partition_id()
            rv16 = rv * BR
            xr = sq_scr[:BR, :]  # f32 scratch is dead after the loop
            nc.sync.dma_start(xr[:], x_ext[ds(rv16, BR), :])
            tr = cpool.tile([BR, 1], F32, tag="tr")
            nc.sync.dma_start(tr[:], t_ext[ds(rv16, BR), :])

            ttr = cpool.tile([BR, 1], F32, tag="ttr")
            nc.vector.tensor_scalar_mul(ttr[:], tr[:], 1.0 / T_SCHEDULE)
            sigr = cpool.tile([BR, 1], F32, tag="sigr")
            nc.vector.tensor_scalar(sigr[:], ttr[:], -1.0, 1.0, OP.mult, OP.add)
            rsigr = cpool.tile([BR, 1], F32, tag="rsigr")
            nc.vector.reciprocal(rsigr[:], sigr[:])
            rs_sc = cpool.tile([BR, 1], F32, tag="rs_sc")
            nc.vector.reciprocal(rs_sc[:], fin[:, D : D + 1])

            o1 = fin[:, :D]  # in-place on the reduce-scatter rows
            nc.vector.scalar_tensor_tensor(
                out=o1[:],
                in0=fin[:, :D],
                scalar=rs_sc[:],
                in1=xr[:],
                op0=OP.mult,
                op1=OP.subtract,
            )
            o2 = y_sb[:BR, :]  # y_sb is dead after y_resc
            nc.vector.tensor_scalar_mul(o2[:], o1[:], rsigr[:])
            nc.sync.dma_start(out_ext[:], o2[:])

    if split_waits:
        _split_multi_waits(nc)
    return nc


def _split_multi_waits(nc):
    """walrus (CoreV2/V3 setupSyncWait) allows one sync wait per engine
    instruction; Tile sometimes emits more.  Hoist all but the last wait onto
    same-queue NoOps inserted immediately before the instruction."""
    n = 0
    for bb in nc.main_func.blocks:
        out = []
        for ins in bb.instructions:
            si = ins.sync_info
            if si is not None and len(si.on_wait) > 1:
                waits = list(si.on_wait)
                for w in waits[:-1]:
                    nop = mybir.InstNoOp(name=f"wait-ladder-{n}")
                    n += 1
                    nop.engine = ins.engine
                    nop.sync_info = mybir.SyncInfo(on_wait=[w], on_update=[])
                    out.append(nop)
                ins.sync_info = mybir.SyncInfo(
                    on_wait=[waits[-1]], on_update=list(si.on_update)
                )
            out.append(ins)
        bb.instructions = out


_NC_CACHE = None


def kernel(xt, t, gt_images):
    global _NC_CACHE
    x = np.ascontiguousarray(xt.reshape(B, -1), dtype=np.float32)
    tcol = np.ascontiguousarray(t.reshape(B, 1), dtype=np.float32)
    gt = np.ascontiguousarray(gt_images.reshape(N, -1), dtype=np.float32)

    if _NC_CACHE is None:
        _NC_CACHE = build_graph()
    nc = _NC_CACHE

    in_maps = []
    for c in range(NCORES):
        shard = np.zeros((NSH_PAD, D), dtype=np.float32)
        shard[:NSH] = gt[c * NSH : (c + 1) * NSH]
        in_maps.append({"x": x, "t": tcol, "gt": shard})
    res = run_bass_kernel_spmd(nc, in_maps, core_ids=list(range(NCORES)))
    out = np.concatenate([res.results[c]["out"] for c in range(NCORES)], axis=0)
    return out.reshape(xt.shape).astype(np.float32)
